# revision 1
# baseline (speedup 1.0000x reference)
"""Multi-head attention (B=4, N=2048, C=1024, H=16, D=64) on 8 trn2 cores.

Sharding: core c = (batch b = c//2, head-half g = c%2). Each core computes
attention for 8 heads of one batch plus the partial output projection over
its 512 channels; the host sums the two partials per batch and adds b_proj.

Device layout (all matmul operands bf16, fp32 PSUM accumulation):
  - host passes xT = x[b].T                       [1024, 2048]
  - QK^T:  qkT[m, n]  = wqk[:, m].T @ xT          (lhsT=wqk, rhs=xT)
  - V:     v[n, vc]   = xT[:, n].T @ wv           (lhsT=xT,  rhs=wv)
           stored interleaved as [V_h | ones] blocks of 65 per head
  - S^T:   s[m, n]    = kT[:, m].T @ qT[:, n]     (per head, contraction d=64)
  - E = exp(s / 8) via ScalarE (scores are O(1): no max subtraction needed)
  - PV:    u[dv, n]   = V1[m, dv].T @ E[m, n]     row 64 = softmax denom
  - norm:  attnT = u[0:64] * broadcast(1/u[64])   (PE K=1 broadcast matmul)
  - proj:  out[n, oc] = attnT[:, n].T @ wp        partial over 512 channels
"""

import numpy as np
import ml_dtypes

B, N, C = 4, 2048, 1024
H, D = 16, 64
HPC = 8            # heads per core
QKC = HPC * D      # 512 q (and k, v) channels per core
NCORES = 8

_nc_cache = None


def build_nc(split_waits=True, repeat=1):
    import concourse.bass as bass
    import concourse.mybir as mybir
    import concourse.tile as tile

    bf16 = mybir.dt.bfloat16
    f32 = mybir.dt.float32
    f32r = mybir.dt.float32r

    nc = bass.Bass()
    xT_d = nc.declare_dram_parameter("xT", [C, N], bf16, isOutput=False)
    wqk_d = nc.declare_dram_parameter("wqk", [C, 2 * QKC], bf16, isOutput=False)
    wv_d = nc.declare_dram_parameter("wv", [C, QKC], bf16, isOutput=False)
    wp_d = nc.declare_dram_parameter("wp", [QKC, C], bf16, isOutput=False)
    out_d = nc.declare_dram_parameter("out", [N, C], f32, isOutput=True)

    rdram = nc.dram_tensor("rscratch", [32, 512], f32)

    KT = C // 128          # 8 contraction tiles for qkv projection
    NT = N // 128          # 16 seq tiles
    NB = N // 512          # 4 seq blocks of 512
    QKT = 2 * QKC // 128   # 8 qk-channel tiles

    with tile.TileContext(nc) as tc:
        with (
            tc.tile_pool(name="big", bufs=1) as big,
            tc.tile_pool(name="work", bufs=18) as workp,
            tc.tile_pool(name="outp", bufs=4) as outp,
            tc.tile_pool(name="small", bufs=4) as smallp,
            tc.tile_pool(name="mm", bufs=2, space="PSUM") as mmp,
            tc.tile_pool(name="spsum", bufs=2, space="PSUM") as spsum,
            tc.tile_pool(name="pvpsum", bufs=2, space="PSUM") as pvpsum,
        ):
            # ---- load inputs ----
            xt = big.tile([128, KT, N], bf16, tag="xt")
            wqk = big.tile([128, KT, 2 * QKC], bf16, tag="wqk")
            wv = big.tile([128, KT, QKC], bf16, tag="wv")
            wp = big.tile([128, QKC // 128, C], bf16, tag="wp")
            xT_r = xT_d.rearrange("(t p) n -> p t n", p=128)
            wqk_r = wqk_d.rearrange("(t p) m -> p t m", p=128)
            wv_r = wv_d.rearrange("(t p) m -> p t m", p=128)
            wp_r = wp_d.rearrange("(t p) m -> p t m", p=128)
            # Each DMA instruction costs ~650ns of serialized issue on the
            # sync sequencer, so use few, large DMAs in consumption order.
            # wqk is host-reordered pair-major ([q|k] 256-col block per head
            # pair) so one DMA loads exactly what head pair 0 needs first.
            # leading chunks split in half so the first qk group's kt=0..3
            # matmuls start while the second half still streams
            nc.sync.dma_start(out=wqk[:, 0:4, 0:256], in_=wqk_r[:, 0:4, 0:256])
            nc.sync.dma_start(out=xt[:, 0:4, 0:512], in_=xT_r[:, 0:4, 0:512])
            nc.sync.dma_start(out=wqk[:, 4:8, 0:256], in_=wqk_r[:, 4:8, 0:256])
            nc.sync.dma_start(out=xt[:, 4:8, 0:512], in_=xT_r[:, 4:8, 0:512])
            for nb in range(1, NB):
                nc.sync.dma_start(out=xt[:, :, nb * 512:(nb + 1) * 512],
                                  in_=xT_r[:, :, nb * 512:(nb + 1) * 512])
            nc.sync.dma_start(out=wv, in_=wv_r)
            for pr in range(1, 4):
                nc.sync.dma_start(out=wqk[:, :, pr * 256:(pr + 1) * 256],
                                  in_=wqk_r[:, :, pr * 256:(pr + 1) * 256])
            nc.sync.dma_start(out=wp, in_=wp_r)

            ones = big.tile([1, 64], bf16, tag="ones")
            nc.vector.memset(ones, 1.0)

            # ---- persistent intermediates ----
            qkT = [big.tile([128, N], bf16, tag=f"qkT{i}", name=f"qkT{i}") for i in range(QKT)]
            v1 = [big.tile([128, HPC * 65], bf16, tag=f"v1_{i}", name=f"v1_{i}") for i in range(NT)]
            attnT = [big.tile([128, N], bf16, tag=f"attnT{i}", name=f"attnT{i}") for i in range(4)]

            def qk_group(mt, nb):
                # wqk is pair-major: q cols of pair p at p*256, k at p*256+128
                co = (mt % 4) * 256 + (mt // 4) * 128
                ps = mmp.tile([128, 512], f32, tag="mm")
                for kt in range(KT):
                    nc.tensor.matmul(
                        ps,
                        lhsT=wqk[:, kt, co:co + 128],
                        rhs=xt[:, kt, nb * 512:(nb + 1) * 512],
                        start=(kt == 0),
                        stop=(kt == KT - 1),
                    )
                nc.vector.tensor_copy(
                    out=qkT[mt][:, nb * 512:(nb + 1) * 512], in_=ps
                )

            def qk_tile(mt):
                """qkT[mt] = (wqk[:, mt*128:+128]).T @ xT  -> [128, 2048]"""
                for nb in range(NB):
                    qk_group(mt, nb)

            def v_tile(nt):
                """v1[nt][:, h*65:h*65+64] = V rows nt*128..; col h*65+64 = 1"""
                ps = mmp.tile([128, 512], f32, tag="mm")
                for kt in range(KT):
                    nc.tensor.matmul(
                        ps,
                        lhsT=xt[:, kt, nt * 128:(nt + 1) * 128],
                        rhs=wv[:, kt, :],
                        start=(kt == 0),
                        stop=(kt == KT - 1),
                    )
                v3 = v1[nt].rearrange("p (h e) -> p h e", e=65)
                nc.vector.memset(v3[:, :, 64:65], 1.0)
                nc.vector.tensor_copy(
                    out=v3[:, :, 0:64],
                    in_=ps.rearrange("p (h e) -> p h e", e=64),
                )

            rb_idx = [0]

            def s_units(h, ng, fillers, dense=False):
                """Generator of 16 S-phase units for block (h, ng): each emits
                the two S matmuls + the exp, plus an optional filler group.
                Appends e tiles to the returned list as units run.  With
                `dense`, one filler is popped at every mt (slot i == mt i, for
                fillers that must land before a specific S/PV consumer)."""
                fillers = list(fillers)
                if dense:
                    slots = set(range(NT))
                elif fillers:
                    stride = max(1, NT // len(fillers))
                    slots = set(range(stride - 1, NT, stride))
                else:
                    slots = set()
                qt = qkT[h // 2]
                kt_ = qkT[4 + h // 2]
                po = (h % 2) * 64
                es = []

                def gen():
                    for mt in range(NT):
                        sp = spsum.tile([128, 1024], f32, tag="sps", name="sp")
                        for half in range(2):
                            nc.tensor.matmul(
                                sp[:, half * 512:(half + 1) * 512],
                                lhsT=kt_[po:po + 64, mt * 128:(mt + 1) * 128],
                                rhs=qt[po:po + 64,
                                       ng * 1024 + half * 512:
                                       ng * 1024 + (half + 1) * 512],
                                start=True,
                                stop=True,
                            )
                        e = workp.tile([128, 1024], bf16, tag="e", name="e")
                        nc.scalar.activation(
                            out=e, in_=sp,
                            func=mybir.ActivationFunctionType.Exp,
                            scale=0.125,
                        )
                        es.append(e)
                        if fillers and mt in slots:
                            fillers.pop(0)()
                        yield
                    while fillers:  # leftovers
                        fillers.pop(0)()

                return es, gen()

            def pv_units(h, ng, es, fillers=(), pe_norm=False):
                """Generator of 32 PV matmul units for block (h, ng); after
                exhaustion emits the two normalization chains.  `fillers` are
                popped one per mt (used to finish V tiles ahead of their PV
                use).  `pe_norm` broadcasts 1/denom with a PE matmul instead
                of the DRAM bounce (shorter latency; used for late blocks on
                the critical path to proj)."""
                fillers = list(fillers)
                po = (h % 2) * 64
                at = attnT[h // 2]
                pvs = [pvpsum.tile([65, 512], f32, tag="pv", name="pv")
                       for _ in range(2)]

                def norm(half):
                    pv = pvs[half]
                    r = smallp.tile([1, 512], bf16, tag="r", name="r")
                    with nc.allow_low_precision(reason="softmax recip bcast"):
                        nc.vector.reciprocal(out=r, in_=pv[64:65, :])
                    # copy the numerator out of PSUM immediately so the pv
                    # slot frees for the next block's PV without waiting for
                    # the broadcast round-trip
                    pvsb = smallp.tile([64, 512], f32, tag="pvsb", name="pvsb")
                    nc.vector.tensor_copy(out=pvsb, in_=pv[0:64, :])
                    rbs = smallp.tile([64, 512], f32, tag="rbs", name="rbs")
                    if pe_norm:
                        rb = mmp.tile([64, 512], f32, tag="mm", name="rb")
                        nc.tensor.matmul(rb, lhsT=ones, rhs=r,
                                         start=True, stop=True)
                        nc.vector.tensor_copy(out=rbs, in_=rb)
                    else:
                        idx = rb_idx[0] % 32
                        rb_idx[0] += 1
                        nc.sync.dma_start(out=rdram[idx], in_=r[0, :])
                        rsl = rdram[idx]
                        bcast = bass.AP(tensor=rsl.tensor, offset=rsl.offset,
                                        ap=[[0, 64]] + [list(p) for p in rsl.ap])
                        nc.sync.dma_start(out=rbs, in_=bcast)
                    nc.vector.tensor_mul(
                        out=at[po:po + 64,
                               ng * 1024 + half * 512:
                               ng * 1024 + (half + 1) * 512],
                        in0=pvsb,
                        in1=rbs,
                    )

                def gen():
                    for mt in range(NT):
                        if fillers:
                            fillers.pop(0)()
                        for half in range(2):
                            nc.tensor.matmul(
                                pvs[half],
                                lhsT=v1[mt][:, h * 65:(h + 1) * 65],
                                rhs=es[mt][:, half * 512:(half + 1) * 512],
                                start=(mt == 0),
                                stop=(mt == NT - 1),
                            )
                            yield
                    norm(0)
                    norm(1)

                return gen()

            def run_all(g):
                for _ in g:
                    pass

            def interleave(sgen, pvgen):
                """2 PV units per S unit (32 PV vs 16 S per block)."""
                while True:
                    done = 0
                    for _ in range(2):
                        if next(pvgen, StopIteration) is StopIteration:
                            done += 1
                            break
                    if next(sgen, StopIteration) is StopIteration:
                        done += 1
                    if done:
                        for _ in pvgen:
                            pass
                        for _ in sgen:
                            pass
                        return

            def proj(nt):
                # the second half of proj runs after the last attention block:
                # rotate over the then-idle pv/sps PSUM slots too, so groups
                # aren't serialized on the two "mm" slots
                if nt < 8:
                    pool_tag = (mmp, "mm")
                else:
                    pool_tag = [(mmp, "mm"), (pvpsum, "pv"), (spsum, "sps")][nt % 3]
                ot = outp.tile([128, C], f32, tag="ot")
                for ob in range(2):
                    ps = pool_tag[0].tile([128, 512], f32, tag=pool_tag[1])
                    for ct in range(QKC // 128):
                        nc.tensor.matmul(
                            ps,
                            lhsT=attnT[ct][:, nt * 128:(nt + 1) * 128],
                            rhs=wp[:, ct, ob * 512:(ob + 1) * 512],
                            start=(ct == 0),
                            stop=(ct == QKC // 128 - 1),
                        )
                    # ScalarE is idle during the projection tail; DVE is not
                    nc.scalar.copy(
                        out=ot[:, ob * 512:(ob + 1) * 512], in_=ps
                    )
                nc.sync.dma_start(
                    out=out_d[nt * 128:(nt + 1) * 128, :], in_=ot
                )

            # Software pipeline over 16 (h, ng) blocks: block i's S-phase (the
            # exp feed) interleaves with block i-1's PV matmuls so ScalarE
            # never starves at head boundaries.  Only qk tiles 0 and 4 precede
            # attention; V tiles are built as fillers inside blocks 0/1, later
            # qk tile-groups inside earlier pairs' blocks (always complete
            # before first use).  The last block's PV overlaps the output
            # projection, and the last two blocks normalize via PE broadcast
            # (short latency) instead of the DRAM bounce.
            import functools
            for _rep in range(repeat):
              # Minimal prelude: S(b0=(h0,ng0), mt) needs q cols 0:1024
              # (qk groups (0,0),(0,1)) and k block nb0 (group (4,0)); the
              # other qk(0)/qk(4) groups and V tiles ride as dense fillers
              # inside block 0, ordered so each lands before its first
              # consumer (group (4,j) before S mt=4j; v1[i] before PV mt i).
              qk_group(0, 0)
              qk_group(0, 1)
              qk_group(4, 0)

              qkg = [[functools.partial(qk_group, m, nb) for nb in range(NB)]
                   for m in range(QKT)]
              vg = [functools.partial(v_tile, nt) for nt in range(NT)]
              block_fill = [[] for _ in range(16)]
              block_fill[0] = [
                  qkg[4][1], qkg[0][2], qkg[0][3], vg[0],
                  qkg[4][2], vg[1], vg[2], vg[3],
                  qkg[4][3], vg[4], vg[5], vg[6],
                  vg[7], vg[8], vg[9], vg[10],
              ]
              pv0_fill = vg[11:16]       # v1[11..15] paced inside PV(b0)
              f15 = qkg[1] + qkg[5]      # tiles 1,5 for head pair 1 (blocks 4-7)
              block_fill[2] = f15[0:4]
              block_fill[3] = f15[4:8]
              f26 = qkg[2] + qkg[6]      # tiles 2,6 for pair 2 (blocks 8-11)
              for i in range(4):
                  block_fill[4 + i] = f26[2 * i:2 * i + 2]
              f37 = qkg[3] + qkg[7]      # tiles 3,7 for pair 3 (blocks 12-15)
              for i in range(4):
                  block_fill[8 + i] = f37[2 * i:2 * i + 2]

              blocks = [(h, ng) for h in range(HPC) for ng in range(2)]
              prev_pv = None
              for bi, (h, ng) in enumerate(blocks):
                  es, sgen = s_units(h, ng, block_fill[bi], dense=(bi == 0))
                  if prev_pv is None:
                      run_all(sgen)
                  else:
                      interleave(sgen, prev_pv)
                  prev_pv = pv_units(
                      h, ng, es,
                      fillers=pv0_fill if bi == 0 else (),
                      pe_norm=True,
                  )

              # tail: last block's PV interleaved with the first half of the
              # projection (those rows need only norm(14), already done); then
              # its norm (PE broadcast, short), then the remaining projection.
              for nt in range(8):
                  for _ in range(4):
                      next(prev_pv, None)
                  proj(nt)
              run_all(prev_pv)
              for nt in range(8, NT):
                  proj(nt)

    if split_waits:
        _split_multi_waits(nc, mybir)
    return nc


def _split_multi_waits(nc, mybir):
    """TPB instructions carry exactly one sync-wait slot; walrus codegen
    rejects instructions Tile scheduled with >1 waits ("Too many sync wait
    commands").  Hoist all but the last wait onto NoOps inserted just before
    the instruction on the same engine queue (queues execute in order, so
    semantics are identical)."""
    eng_ok = {
        mybir.EngineType.PE,
        mybir.EngineType.Activation,
        mybir.EngineType.DVE,
        mybir.EngineType.Pool,
        mybir.EngineType.SP,
    }
    k = 0
    for f in nc.m.functions:
        for blk in f.blocks:
            out = []
            changed = False
            for inst in blk.instructions:
                si = inst.sync_info
                if (
                    si is not None
                    and len(si.on_wait) > 1
                    and inst.engine in eng_ok
                ):
                    waits = list(si.on_wait)
                    for w in waits[:-1]:
                        nop = mybir.InstNoOp(name=f"I-splitw-{k}", ins=[], outs=[])
                        k += 1
                        nop.engine = inst.engine
                        nop.sync_info = mybir.SyncInfo(on_wait=[w], on_update=[])
                        out.append(nop)
                    inst.sync_info = mybir.SyncInfo(
                        on_wait=[waits[-1]], on_update=list(si.on_update)
                    )
                    changed = True
                out.append(inst)
            if changed:
                blk.instructions = out


def _get_nc():
    global _nc_cache
    if _nc_cache is None:
        _nc_cache = build_nc()
    return _nc_cache


def make_in_maps(x, W_qkv, W_proj):
    bf16 = ml_dtypes.bfloat16
    in_maps = []
    for c in range(NCORES):
        b, g = divmod(c, 2)
        xT = np.ascontiguousarray(np.asarray(x[b]).T).astype(bf16)
        wq = W_qkv[:, g * QKC:(g + 1) * QKC]
        wk = W_qkv[:, C + g * QKC:C + (g + 1) * QKC]
        # pair-major: [q128 | k128] per head pair, matching qk_group's co map
        wqk = np.concatenate(
            [blk for p in range(4)
             for blk in (wq[:, p * 128:(p + 1) * 128],
                         wk[:, p * 128:(p + 1) * 128])],
            axis=1,
        ).astype(bf16)
        wv = np.ascontiguousarray(W_qkv[:, 2 * C + g * QKC:2 * C + (g + 1) * QKC]).astype(bf16)
        wp = np.ascontiguousarray(W_proj[g * QKC:(g + 1) * QKC, :]).astype(bf16)
        in_maps.append({"xT": xT, "wqk": wqk, "wv": wv, "wp": wp})
    return in_maps


last_exec_time_ns = None


def kernel(x, W_qkv, W_proj, b_proj):
    global last_exec_time_ns
    import os
    # the NTFF trace path needs antenv.axon_hooks, absent in this container
    os.environ["BASS_NEVER_TRACE"] = "1"
    from concourse import bass_utils

    x = np.asarray(x)
    W_qkv = np.asarray(W_qkv)
    W_proj = np.asarray(W_proj)
    b_proj = np.asarray(b_proj)

    nc = _get_nc()
    in_maps = make_in_maps(x, W_qkv, W_proj)
    res = bass_utils.run_bass_kernel_spmd(nc, in_maps, list(range(NCORES)))
    last_exec_time_ns = res.exec_time_ns

    out = np.empty((B, N, C), np.float32)
    bias = b_proj.astype(np.float32)
    for b in range(B):
        out[b] = res.results[2 * b]["out"] + res.results[2 * b + 1]["out"] + bias
    return out



# revision 17
# speedup vs baseline: 2681.5241x; 2681.5241x over previous
"""Multi-head attention (B=4, N=2048, C=1024, H=16, D=64) on 8 trn2 cores.

Sharding: core c = (batch b = c//2, head-half g = c%2). Each core computes
attention for 8 heads of one batch plus the partial output projection over
its 512 channels; the host sums the two partials per batch and adds b_proj.

Device layout (matmul operands bf16 except S in fp8, fp32 PSUM accum):
  - host passes xT = x[b].T                       [1024, 2048]
  - QK^T:  qkT[m, n]  = wqk[:, m].T @ xT          (lhsT=wqk, rhs=xT)
           stored as fp8e4m3 (feeds only the S matmul)
  - V:     v[n, vc]   = xT[:, n].T @ wv           (lhsT=xT,  rhs=wv)
           stored interleaved as [V_h | ones] blocks of 65 per head
  - S^T:   s[m, n]    = kT[:, m].T @ qT[:, n]     (per head, contraction d=64)
           fp8 DoubleRow at 0.5 cyc/col: a stride-0 j-dim reads each
           operand twice, so the PE computes 2*K^T Q; the doubling is
           absorbed into the exp scale (0.0625 instead of 0.125)
  - E = exp(2s/16) via ScalarE (scores are O(1): no max subtraction needed)
  - PV:    u[dv, n]   = V1[m, dv].T @ E[m, n]     row 64 = softmax denom
  - norm:  attnT = u[0:64] * broadcast(1/u[64])   (GpSimd partition bcast)
  - proj:  out[n, oc] = attnT[:, n].T @ wp        partial over 512 channels
"""

import numpy as np
import ml_dtypes

B, N, C = 4, 2048, 1024
H, D = 16, 64
HPC = 8            # heads per core
QKC = HPC * D      # 512 q (and k, v) channels per core
NCORES = 8

_nc_cache = None


def build_nc(split_waits=True, repeat=1):
    import concourse.bass as bass
    import concourse.mybir as mybir
    import concourse.tile as tile

    bf16 = mybir.dt.bfloat16
    fp8 = mybir.dt.float8e4
    f32 = mybir.dt.float32
    f32r = mybir.dt.float32r

    nc = bass.Bass()
    xT_d = nc.declare_dram_parameter("xT", [C, N], bf16, isOutput=False)
    wqk_d = nc.declare_dram_parameter("wqk", [C, 2 * QKC], bf16, isOutput=False)
    wv_d = nc.declare_dram_parameter("wv", [C, QKC], bf16, isOutput=False)
    wp_d = nc.declare_dram_parameter("wp", [QKC, C], bf16, isOutput=False)
    out_d = nc.declare_dram_parameter("out", [N, C], f32, isOutput=True)

    rdram = nc.dram_tensor("rscratch", [32, 512], bf16)

    KT = C // 128          # 8 contraction tiles for qkv projection
    NT = N // 128          # 16 seq tiles
    NB = N // 512          # 4 seq blocks of 512
    QKT = 2 * QKC // 128   # 8 qk-channel tiles

    with tile.TileContext(nc) as tc:
        with (
            tc.tile_pool(name="big", bufs=1) as big,
            tc.tile_pool(name="work", bufs=18) as workp,
            tc.tile_pool(name="outp", bufs=4) as outp,
            tc.tile_pool(name="small", bufs=4) as smallp,
            tc.tile_pool(name="mm", bufs=2, space="PSUM") as mmp,
            tc.tile_pool(name="spsum", bufs=2, space="PSUM") as spsum,
            tc.tile_pool(name="pvpsum", bufs=2, space="PSUM") as pvpsum,
        ):
            # ---- load inputs ----
            xt = big.tile([128, KT, N], bf16, tag="xt")
            wqk = big.tile([128, KT, 2 * QKC], bf16, tag="wqk")
            wv = big.tile([128, KT, QKC], bf16, tag="wv")
            wp = big.tile([128, QKC // 128, C], bf16, tag="wp")
            xT_r = xT_d.rearrange("(t p) n -> p t n", p=128)
            wqk_r = wqk_d.rearrange("(t p) m -> p t m", p=128)
            wv_r = wv_d.rearrange("(t p) m -> p t m", p=128)
            wp_r = wp_d.rearrange("(t p) m -> p t m", p=128)
            # Each DMA instruction costs ~650ns of serialized issue on the
            # sync sequencer, so use few, large DMAs in consumption order.
            # wqk is host-reordered pair-major ([q|k] 256-col block per head
            # pair) so one DMA loads exactly what head pair 0 needs first.
            # leading chunks split in half so the first qk group's kt=0..3
            # matmuls start while the second half still streams
            nc.sync.dma_start(out=wqk[:, 0:4, 0:256], in_=wqk_r[:, 0:4, 0:256])
            nc.sync.dma_start(out=xt[:, 0:4, 0:512], in_=xT_r[:, 0:4, 0:512])
            nc.sync.dma_start(out=wqk[:, 4:8, 0:256], in_=wqk_r[:, 4:8, 0:256])
            nc.sync.dma_start(out=xt[:, 4:8, 0:512], in_=xT_r[:, 4:8, 0:512])
            for nb in range(1, NB):
                nc.sync.dma_start(out=xt[:, :, nb * 512:(nb + 1) * 512],
                                  in_=xT_r[:, :, nb * 512:(nb + 1) * 512])
            nc.sync.dma_start(out=wv, in_=wv_r)
            for pr in range(1, 4):
                nc.sync.dma_start(out=wqk[:, :, pr * 256:(pr + 1) * 256],
                                  in_=wqk_r[:, :, pr * 256:(pr + 1) * 256])
            nc.sync.dma_start(out=wp, in_=wp_r)

            ones = big.tile([1, 64], bf16, tag="ones")
            nc.vector.memset(ones, 1.0)

            # ---- persistent intermediates ----
            qkT = [big.tile([128, N], fp8, tag=f"qkT{i}", name=f"qkT{i}") for i in range(QKT)]
            v1 = [big.tile([128, HPC * 65], bf16, tag=f"v1_{i}", name=f"v1_{i}") for i in range(NT)]
            attnT = [big.tile([128, N], bf16, tag=f"attnT{i}", name=f"attnT{i}") for i in range(4)]

            def qk_group(mt, nb):
                # wqk is pair-major: q cols of pair p at p*256, k at p*256+128
                co = (mt % 4) * 256 + (mt // 4) * 128
                ps = mmp.tile([128, 512], f32, tag="mm")
                for kt in range(KT):
                    nc.tensor.matmul(
                        ps,
                        lhsT=wqk[:, kt, co:co + 128],
                        rhs=xt[:, kt, nb * 512:(nb + 1) * 512],
                        start=(kt == 0),
                        stop=(kt == KT - 1),
                    )
                nc.vector.tensor_copy(
                    out=qkT[mt][:, nb * 512:(nb + 1) * 512], in_=ps
                )

            def qk_tile(mt):
                """qkT[mt] = (wqk[:, mt*128:+128]).T @ xT  -> [128, 2048]"""
                for nb in range(NB):
                    qk_group(mt, nb)

            def v_tile(nt):
                """v1[nt][:, h*65:h*65+64] = V rows nt*128..; col h*65+64 = 1"""
                ps = mmp.tile([128, 512], f32, tag="mm")
                for kt in range(KT):
                    nc.tensor.matmul(
                        ps,
                        lhsT=xt[:, kt, nt * 128:(nt + 1) * 128],
                        rhs=wv[:, kt, :],
                        start=(kt == 0),
                        stop=(kt == KT - 1),
                    )
                v3 = v1[nt].rearrange("p (h e) -> p h e", e=65)
                nc.vector.memset(v3[:, :, 64:65], 1.0)
                nc.vector.tensor_copy(
                    out=v3[:, :, 0:64],
                    in_=ps.rearrange("p (h e) -> p h e", e=64),
                )

            rb_idx = [0]

            def dr2(sl):
                """Insert a stride-0 j-dim of size 2 after the partition dim:
                DoubleRow reads the operand twice (result is doubled)."""
                return bass.AP(
                    tensor=sl.tensor, offset=sl.offset,
                    ap=[list(sl.ap[0]), [0, 2]] + [list(p) for p in sl.ap[1:]],
                )

            def s_units(h, ng, fillers, dense=False):
                """Generator of 16 S-phase units for block (h, ng): each emits
                the two S matmuls + the exp, plus an optional filler group.
                Appends e tiles to the returned list as units run.  With
                `dense`, one filler is popped at every mt (slot i == mt i, for
                fillers that must land before a specific S/PV consumer)."""
                fillers = list(fillers)
                if dense:
                    slots = set(range(NT))
                elif fillers:
                    stride = max(1, NT // len(fillers))
                    slots = set(range(stride - 1, NT, stride))
                else:
                    slots = set()
                qt = qkT[h // 2]
                kt_ = qkT[4 + h // 2]
                po = (h % 2) * 64
                es = []

                def gen():
                    for mt in range(NT):
                        sp = spsum.tile([128, 1024], f32, tag="sps", name="sp")
                        for half in range(2):
                            nc.tensor.matmul(
                                sp[:, half * 512:(half + 1) * 512],
                                lhsT=dr2(kt_[po:po + 64, mt * 128:(mt + 1) * 128]),
                                rhs=dr2(qt[po:po + 64,
                                           ng * 1024 + half * 512:
                                           ng * 1024 + (half + 1) * 512]),
                                start=True,
                                stop=True,
                                perf_mode=mybir.MatmulPerfMode.DoubleRow,
                            )
                        e = workp.tile([128, 1024], bf16, tag="e", name="e")
                        nc.scalar.activation(
                            out=e, in_=sp,
                            func=mybir.ActivationFunctionType.Exp,
                            scale=0.0625,
                        )
                        es.append(e)
                        if fillers and mt in slots:
                            fillers.pop(0)()
                        yield
                    while fillers:  # leftovers
                        fillers.pop(0)()

                return es, gen()

            def pv_units(h, ng, es, fillers=(), pe_norm=False):
                """Generator of 32 PV matmul units for block (h, ng); after
                exhaustion emits the two normalization chains.  `fillers` are
                popped one per mt (used to finish V tiles ahead of their PV
                use).  `pe_norm` broadcasts 1/denom with a PE matmul instead
                of the DRAM bounce (shorter latency; used for late blocks on
                the critical path to proj)."""
                fillers = list(fillers)
                po = (h % 2) * 64
                at = attnT[h // 2]
                pvs = [pvpsum.tile([65, 512], f32, tag="pv", name="pv")
                       for _ in range(2)]

                def norm(half):
                    pv = pvs[half]
                    r = smallp.tile([1, 512], bf16, tag="r", name="r")
                    with nc.allow_low_precision(reason="softmax recip bcast"):
                        nc.vector.reciprocal(out=r, in_=pv[64:65, :])
                    # copy the numerator out of PSUM immediately so the pv
                    # slot frees for the next block's PV without waiting for
                    # the broadcast round-trip
                    pvsb = smallp.tile([64, 512], f32, tag="pvsb", name="pvsb")
                    nc.vector.tensor_copy(out=pvsb, in_=pv[0:64, :])
                    if pe_norm:
                        rbs = smallp.tile([64, 512], f32, tag="rbs", name="rbs")
                        rb = mmp.tile([64, 512], f32, tag="mm", name="rb")
                        nc.tensor.matmul(rb, lhsT=ones, rhs=r,
                                         start=True, stop=True)
                        nc.vector.tensor_copy(out=rbs, in_=rb)
                    else:
                        rbs = smallp.tile([64, 512], bf16, tag="rbs", name="rbs")
                        idx = rb_idx[0] % 32
                        rb_idx[0] += 1
                        nc.sync.dma_start(out=rdram[idx], in_=r[0, :])
                        rsl = rdram[idx]
                        bcast = bass.AP(tensor=rsl.tensor, offset=rsl.offset,
                                        ap=[[0, 64]] + [list(p) for p in rsl.ap])
                        nc.sync.dma_start(out=rbs, in_=bcast)
                    nc.vector.tensor_mul(
                        out=at[po:po + 64,
                               ng * 1024 + half * 512:
                               ng * 1024 + (half + 1) * 512],
                        in0=pvsb,
                        in1=rbs,
                    )

                def gen():
                    for mt in range(NT):
                        if fillers:
                            fillers.pop(0)()
                        for half in range(2):
                            nc.tensor.matmul(
                                pvs[half],
                                lhsT=v1[mt][:, h * 65:(h + 1) * 65],
                                rhs=es[mt][:, half * 512:(half + 1) * 512],
                                start=(mt == 0),
                                stop=(mt == NT - 1),
                            )
                            yield
                    norm(0)
                    norm(1)

                return gen()

            def run_all(g):
                for _ in g:
                    pass

            def interleave(sgen, pvgen):
                """2 PV units per S unit (32 PV vs 16 S per block)."""
                while True:
                    done = 0
                    for _ in range(2):
                        if next(pvgen, StopIteration) is StopIteration:
                            done += 1
                            break
                    if next(sgen, StopIteration) is StopIteration:
                        done += 1
                    if done:
                        for _ in pvgen:
                            pass
                        for _ in sgen:
                            pass
                        return

            def proj(nt):
                # the second half of proj runs after the last attention block:
                # rotate over the then-idle pv/sps PSUM slots too, so groups
                # aren't serialized on the two "mm" slots
                if nt < 8:
                    pool_tag = (mmp, "mm")
                else:
                    pool_tag = [(mmp, "mm"), (pvpsum, "pv"), (spsum, "sps")][nt % 3]
                ot = outp.tile([128, C], f32, tag="ot")
                for ob in range(2):
                    ps = pool_tag[0].tile([128, 512], f32, tag=pool_tag[1])
                    for ct in range(QKC // 128):
                        nc.tensor.matmul(
                            ps,
                            lhsT=attnT[ct][:, nt * 128:(nt + 1) * 128],
                            rhs=wp[:, ct, ob * 512:(ob + 1) * 512],
                            start=(ct == 0),
                            stop=(ct == QKC // 128 - 1),
                        )
                    # ScalarE is idle during the projection tail; DVE is not
                    nc.scalar.copy(
                        out=ot[:, ob * 512:(ob + 1) * 512], in_=ps
                    )
                nc.sync.dma_start(
                    out=out_d[nt * 128:(nt + 1) * 128, :], in_=ot
                )

            # Software pipeline over 16 (h, ng) blocks: block i's S-phase (the
            # exp feed) interleaves with block i-1's PV matmuls so ScalarE
            # never starves at head boundaries.  Only qk tiles 0 and 4 precede
            # attention; V tiles are built as fillers inside blocks 0/1, later
            # qk tile-groups inside earlier pairs' blocks (always complete
            # before first use).  The last block's PV overlaps the output
            # projection, and the last two blocks normalize via PE broadcast
            # (short latency) instead of the DRAM bounce.
            import functools
            for _rep in range(repeat):
              # Minimal prelude: S(b0=(h0,ng0), mt) needs q cols 0:1024
              # (qk groups (0,0),(0,1)) and k block nb0 (group (4,0)); the
              # other qk(0)/qk(4) groups and V tiles ride as dense fillers
              # inside block 0, ordered so each lands before its first
              # consumer (group (4,j) before S mt=4j; v1[i] before PV mt i).
              qk_group(0, 0)
              qk_group(0, 1)
              qk_group(4, 0)

              qkg = [[functools.partial(qk_group, m, nb) for nb in range(NB)]
                   for m in range(QKT)]
              vg = [functools.partial(v_tile, nt) for nt in range(NT)]
              block_fill = [[] for _ in range(16)]
              block_fill[0] = [
                  qkg[4][1], qkg[0][2], qkg[0][3], vg[0],
                  qkg[4][2], vg[1], vg[2], vg[3],
                  qkg[4][3], vg[4], vg[5], vg[6],
                  vg[7], vg[8], vg[9], vg[10],
              ]
              pv0_fill = vg[11:16]       # v1[11..15] paced inside PV(b0)
              f15 = qkg[1] + qkg[5]      # tiles 1,5 for head pair 1 (blocks 4-7)
              block_fill[2] = f15[0:4]
              block_fill[3] = f15[4:8]
              f26 = qkg[2] + qkg[6]      # tiles 2,6 for pair 2 (blocks 8-11)
              for i in range(4):
                  block_fill[4 + i] = f26[2 * i:2 * i + 2]
              f37 = qkg[3] + qkg[7]      # tiles 3,7 for pair 3 (blocks 12-15)
              for i in range(4):
                  block_fill[8 + i] = f37[2 * i:2 * i + 2]

              blocks = [(h, ng) for h in range(HPC) for ng in range(2)]
              prev_pv = None
              for bi, (h, ng) in enumerate(blocks):
                  es, sgen = s_units(h, ng, block_fill[bi], dense=(bi == 0))
                  if prev_pv is None:
                      run_all(sgen)
                  else:
                      interleave(sgen, prev_pv)
                  prev_pv = pv_units(
                      h, ng, es,
                      fillers=pv0_fill if bi == 0 else (),
                      pe_norm=True,
                  )

              # tail: last block's PV interleaved with the first half of the
              # projection (those rows need only norm(14), already done); then
              # its norm (PE broadcast, short), then the remaining projection.
              for nt in range(8):
                  for _ in range(4):
                      next(prev_pv, None)
                  proj(nt)
              run_all(prev_pv)
              for nt in range(8, NT):
                  proj(nt)

    if split_waits:
        _split_multi_waits(nc, mybir)
    return nc


def _split_multi_waits(nc, mybir):
    """TPB instructions carry exactly one sync-wait slot; walrus codegen
    rejects instructions Tile scheduled with >1 waits ("Too many sync wait
    commands").  Hoist all but the last wait onto NoOps inserted just before
    the instruction on the same engine queue (queues execute in order, so
    semantics are identical)."""
    eng_ok = {
        mybir.EngineType.PE,
        mybir.EngineType.Activation,
        mybir.EngineType.DVE,
        mybir.EngineType.Pool,
        mybir.EngineType.SP,
    }
    k = 0
    for f in nc.m.functions:
        for blk in f.blocks:
            out = []
            changed = False
            for inst in blk.instructions:
                si = inst.sync_info
                if (
                    si is not None
                    and len(si.on_wait) > 1
                    and inst.engine in eng_ok
                ):
                    waits = list(si.on_wait)
                    for w in waits[:-1]:
                        nop = mybir.InstNoOp(name=f"I-splitw-{k}", ins=[], outs=[])
                        k += 1
                        nop.engine = inst.engine
                        nop.sync_info = mybir.SyncInfo(on_wait=[w], on_update=[])
                        out.append(nop)
                    inst.sync_info = mybir.SyncInfo(
                        on_wait=[waits[-1]], on_update=list(si.on_update)
                    )
                    changed = True
                out.append(inst)
            if changed:
                blk.instructions = out


def _get_nc():
    global _nc_cache
    if _nc_cache is None:
        _nc_cache = build_nc()
    return _nc_cache


def make_in_maps(x, W_qkv, W_proj):
    bf16 = ml_dtypes.bfloat16
    in_maps = []
    for c in range(NCORES):
        b, g = divmod(c, 2)
        xT = np.ascontiguousarray(np.asarray(x[b]).T).astype(bf16)
        wq = W_qkv[:, g * QKC:(g + 1) * QKC]
        wk = W_qkv[:, C + g * QKC:C + (g + 1) * QKC]
        # pair-major: [q128 | k128] per head pair, matching qk_group's co map
        wqk = np.concatenate(
            [blk for p in range(4)
             for blk in (wq[:, p * 128:(p + 1) * 128],
                         wk[:, p * 128:(p + 1) * 128])],
            axis=1,
        ).astype(bf16)
        wv = np.ascontiguousarray(W_qkv[:, 2 * C + g * QKC:2 * C + (g + 1) * QKC]).astype(bf16)
        wp = np.ascontiguousarray(W_proj[g * QKC:(g + 1) * QKC, :]).astype(bf16)
        in_maps.append({"xT": xT, "wqk": wqk, "wv": wv, "wp": wp})
    return in_maps


last_exec_time_ns = None


def kernel(x, W_qkv, W_proj, b_proj):
    global last_exec_time_ns
    import os
    # the NTFF trace path needs antenv.axon_hooks, absent in this container
    os.environ["BASS_NEVER_TRACE"] = "1"
    from concourse import bass_utils

    x = np.asarray(x)
    W_qkv = np.asarray(W_qkv)
    W_proj = np.asarray(W_proj)
    b_proj = np.asarray(b_proj)

    nc = _get_nc()
    in_maps = make_in_maps(x, W_qkv, W_proj)
    res = bass_utils.run_bass_kernel_spmd(nc, in_maps, list(range(NCORES)))
    last_exec_time_ns = res.exec_time_ns

    out = np.empty((B, N, C), np.float32)
    bias = b_proj.astype(np.float32)
    for b in range(B):
        out[b] = res.results[2 * b]["out"] + res.results[2 * b + 1]["out"] + bias
    return out

